# revision 5
# baseline (speedup 1.0000x reference)
"""Multi-headed attention (B=2, L=2048, E=1024, H=16) on 8 trn2 cores.

Sharding: batch (2) x head-groups (4) -> 8 cores. Each core computes 4 heads
of one batch element end-to-end (QKV projection, attention, partial output
projection); host sums the 4 per-head-group partial outputs per batch and
adds the final bias.

All matmuls run in float32r (full-rate fp32 with reduced internal precision).
Layout choices avoid every transpose except V (PE-transposed once):
  - Q^T, K^T computed directly as [e, l] (head dim on partitions).
  - Scores computed k-major (S^T), exp'd on ACT straight out of PSUM.
  - V stored interleaved [k, 4*(64+1)] with a ones column per head, so the
    PV matmul also accumulates the softmax denominators for free.
  - ctx^T [e, l] is exactly the lhsT the output projection needs.
"""

import numpy as np

EMBED = 1024
HEADS = 16
HD = 64
B = 2
L = 2048
N_CORES = 8
HPC = 4              # heads per core
ES = HPC * HD        # 256: e-slice width per core
NEC = EMBED // 128   # 8 embed chunks
NQC = L // 512       # 4 q-chunks
NKT = L // 128       # 16 k-tiles
VW = HPC * (HD + 1)  # 260: interleaved V width

_CACHE = {}


def _gen_kernel():
    from contextlib import ExitStack

    import concourse.bass as bass
    import concourse.mybir as mybir
    import concourse.tile as tile
    from concourse import bacc
    from concourse.masks import make_identity

    dt = mybir.dt
    f32 = dt.float32
    f32r = dt.float32r

    nc = bacc.Bacc("TRN2", target_bir_lowering=False)

    xqT = nc.dram_tensor("xqT", [EMBED, L], f32, kind="ExternalInput")
    xkT = nc.dram_tensor("xkT", [EMBED, L], f32, kind="ExternalInput")
    xvT = nc.dram_tensor("xvT", [EMBED, L], f32, kind="ExternalInput")
    wT = nc.dram_tensor("wT", [EMBED, ES], f32, kind="ExternalInput")
    woT = nc.dram_tensor("woT", [ES, EMBED], f32, kind="ExternalInput")
    bqkv = nc.dram_tensor("bqkv", [128, 2], f32, kind="ExternalInput")
    vbias = nc.dram_tensor("vbias", [128, ES], f32, kind="ExternalInput")
    onesb = nc.dram_tensor("onesb", [128, 64], f32, kind="ExternalInput")
    out = nc.dram_tensor("out", [L, EMBED], f32, kind="ExternalOutput")

    with tile.TileContext(nc) as tc, ExitStack() as ctx:
        const = ctx.enter_context(tc.tile_pool(name="const", bufs=1))
        stage = ctx.enter_context(tc.tile_pool(name="stage", bufs=3))
        xrnd = ctx.enter_context(tc.tile_pool(name="xrnd", bufs=12))
        big = ctx.enter_context(tc.tile_pool(name="big", bufs=1))
        ptp = ctx.enter_context(tc.tile_pool(name="ptp", bufs=4))
        misc = ctx.enter_context(tc.tile_pool(name="misc", bufs=2))
        opool = ctx.enter_context(tc.tile_pool(name="opool", bufs=3))
        pp_mm = ctx.enter_context(tc.tile_pool(name="pp_mm", bufs=3, space="PSUM"))
        pp_s = ctx.enter_context(tc.tile_pool(name="pp_s", bufs=2, space="PSUM"))
        pp_ctx = ctx.enter_context(tc.tile_pool(name="pp_ctx", bufs=2, space="PSUM"))
        pp_bc = ctx.enter_context(tc.tile_pool(name="pp_bc", bufs=1, space="PSUM"))

        # ---- constants ----------------------------------------------------
        wt_f = stage.tile([128, NEC * ES], f32, tag="wstage")
        for c in range(NEC):
            nc.sync.dma_start(wt_f[:, c * ES:(c + 1) * ES], wT[c * 128:(c + 1) * 128, :])
        wt_r = const.tile([128, NEC * ES], f32r)
        nc.vector.tensor_copy(wt_r[:], wt_f[:])

        wo_f = stage.tile([128, 2 * EMBED], f32, tag="wstage")
        for g in range(2):
            nc.sync.dma_start(wo_f[:, g * EMBED:(g + 1) * EMBED], woT[g * 128:(g + 1) * 128, :])
        wo_r = const.tile([128, 2 * EMBED], f32r)
        nc.vector.tensor_copy(wo_r[:], wo_f[:])

        bq = const.tile([128, 2], f32)
        nc.sync.dma_start(bq[:], bqkv[:])
        vb = const.tile([128, ES], f32)
        nc.sync.dma_start(vb[:], vbias[:])
        ones_f = const.tile([128, 64], f32)
        nc.sync.dma_start(ones_f[:], onesb[:])
        ones_r = const.tile([128, 64], f32r)
        nc.vector.tensor_copy(ones_r[:], ones_f[:])

        idn = const.tile([128, 128], f32)
        make_identity(nc, idn[:])

        # ---- persistent activations --------------------------------------
        qt = big.tile([128, 2 * L], f32r)     # Q^T: e-group g at free g*L
        ktt = big.tile([128, 2 * L], f32r)    # K^T
        vtt = big.tile([128, 2 * L], f32)     # V^T staging (fp32)
        vaug = big.tile([128, NKT * VW], f32r)  # interleaved V + ones cols
        ctxT = big.tile([128, 2 * L], f32r)

        # ---- stage 1: QKV projections ------------------------------------
        # dest^T[e, l] = sum_emb W1s[e, emb] * x[l, emb]; lhsT = wT chunks,
        # rhs = x^T chunks streamed from DRAM (rounded to f32r on DVE).
        for xdram, dest, is_v in ((xqT, qt, False), (xkT, ktt, False), (xvT, vtt, True)):
            for qc in range(NQC):
                blocks = []
                for c in range(NEC):
                    bf = stage.tile([128, 512], f32, tag="xblk")
                    nc.sync.dma_start(
                        bf[:], xdram[c * 128:(c + 1) * 128, qc * 512:(qc + 1) * 512])
                    br = xrnd.tile([128, 512], f32r, tag="xrnd")
                    nc.vector.tensor_copy(br[:], bf[:])
                    blocks.append(br)
                for g in range(2):
                    ps = pp_mm.tile([128, 512], f32, tag="mm")
                    for c in range(NEC):
                        nc.tensor.matmul(
                            ps[:],
                            lhsT=wt_r[:, c * ES + g * 128: c * ES + (g + 1) * 128],
                            rhs=blocks[c][:],
                            start=(c == 0),
                            stop=(c == NEC - 1),
                        )
                    sl = (slice(None), slice(g * L + qc * 512, g * L + (qc + 1) * 512))
                    if is_v:
                        nc.vector.tensor_scalar_add(vtt[sl], ps[:], bq[:, g:g + 1])
                    else:
                        nc.vector.tensor_scalar_add(dest[sl], ps[:], bq[:, g:g + 1])

        # ---- V^T -> interleaved V ----------------------------------------
        for t in range(NKT):
            for g in range(2):
                tp = pp_s.tile([128, 512], f32, tag="s")
                nc.tensor.transpose(
                    tp[:, 0:128], vtt[:, g * L + t * 128: g * L + (t + 1) * 128], idn[:])
                # scatter the two heads of this group into their 65-wide slots
                o0 = t * VW + (2 * g) * (HD + 1)
                dst = vaug[:, o0: o0 + 2 * (HD + 1)].rearrange(
                    "p (a b) -> p a b", b=HD + 1)[:, :, 0:HD]
                nc.vector.tensor_add(
                    dst,
                    tp[:, 0:128].rearrange("p (a b) -> p a b", a=2),
                    vb[:, g * 128:(g + 1) * 128].rearrange("p (a b) -> p a b", a=2),
                )
        # ones columns for all 4 heads of all 16 tiles, one strided copy
        ones_dst = vaug[:].rearrange(
            "p (t h x) -> p x (t h)", h=HPC, x=HD + 1)[:, HD:HD + 1, :]
        nc.vector.tensor_copy(
            ones_dst, ones_f[:, 0:NKT * HPC].rearrange("p (a b) -> p a b", a=1))

        # ---- stage 2: attention ------------------------------------------
        inv_sqrt_e = 1.0 / 32.0
        for h in range(HPC):
            g = h // 2
            off = (h % 2) * 64
            for qc in range(NQC):
                cps = pp_ctx.tile([128, 512], f32, tag="ctx")
                for kt in range(NKT):
                    sps = pp_s.tile([128, 512], f32, tag="s")
                    nc.tensor.matmul(
                        sps[:],
                        lhsT=ktt[off:off + 64, g * L + kt * 128: g * L + (kt + 1) * 128],
                        rhs=qt[off:off + 64, g * L + qc * 512: g * L + (qc + 1) * 512],
                        start=True,
                        stop=True,
                    )
                    pt = ptp.tile([128, 512], f32r, tag="pt")
                    nc.scalar.activation(
                        pt[:], sps[:], mybir.ActivationFunctionType.Exp,
                        scale=inv_sqrt_e)
                    nc.tensor.matmul(
                        cps[0:65, :],
                        lhsT=vaug[:, kt * VW + h * (HD + 1): kt * VW + (h + 1) * (HD + 1)],
                        rhs=pt[:],
                        start=(kt == 0),
                        stop=(kt == NKT - 1),
                    )
                # normalize: rows 0:64 are ctx^T, row 64 is the denominator
                rec = misc.tile([128, 512], f32, tag="rec")
                nc.vector.reciprocal(rec[0:1, :], cps[64:65, :])
                recr = misc.tile([128, 512], f32r, tag="recr")
                nc.vector.tensor_copy(recr[0:1, :], rec[0:1, :])
                bps = pp_bc.tile([128, 512], f32, tag="bc")
                nc.tensor.matmul(
                    bps[0:64, :], lhsT=ones_r[0:1, 0:64], rhs=recr[0:1, :],
                    start=True, stop=True)
                bcs = misc.tile([128, 512], f32, tag="bcs")
                nc.vector.tensor_copy(bcs[0:64, :], bps[0:64, :])
                nc.vector.tensor_mul(
                    ctxT[off:off + 64, g * L + qc * 512: g * L + (qc + 1) * 512],
                    cps[0:64, :],
                    bcs[0:64, :],
                )

        # ---- stage 3: output projection ----------------------------------
        for lt in range(NKT):
            for oc in range(2):
                ops = pp_mm.tile([128, 512], f32, tag="mm")
                for g in range(2):
                    nc.tensor.matmul(
                        ops[:],
                        lhsT=ctxT[:, g * L + lt * 128: g * L + (lt + 1) * 128],
                        rhs=wo_r[:, g * EMBED + oc * 512: g * EMBED + (oc + 1) * 512],
                        start=(g == 0),
                        stop=(g == 1),
                    )
                ot = opool.tile([128, 512], f32, tag="ot")
                nc.scalar.copy(ot[:], ops[:])
                nc.sync.dma_start(
                    out[lt * 128:(lt + 1) * 128, oc * 512:(oc + 1) * 512], ot[:])

    nc.compile()
    return nc


def kernel(query, key, values, W1, b1):
    from concourse.bass_utils import run_bass_kernel_spmd

    if "nc" not in _CACHE:
        _CACHE["nc"] = _gen_kernel()
    nc = _CACHE["nc"]

    query = np.asarray(query, dtype=np.float32)
    key = np.asarray(key, dtype=np.float32)
    values = np.asarray(values, dtype=np.float32)
    W1 = np.asarray(W1, dtype=np.float32)
    b1 = np.asarray(b1, dtype=np.float32)

    xT = {}
    for b in range(B):
        xT[("q", b)] = np.ascontiguousarray(query[b].T)
        xT[("k", b)] = np.ascontiguousarray(key[b].T)
        xT[("v", b)] = np.ascontiguousarray(values[b].T)

    onesb = np.ones((128, 64), dtype=np.float32)
    in_maps = []
    for core in range(N_CORES):
        b = core // HPC
        hg = core % HPC
        sl = slice(hg * ES, (hg + 1) * ES)
        in_maps.append({
            "xqT": xT[("q", b)],
            "xkT": xT[("k", b)],
            "xvT": xT[("v", b)],
            "wT": np.ascontiguousarray(W1[sl, :].T),
            "woT": np.ascontiguousarray(W1[:, sl].T),
            "bqkv": np.ascontiguousarray(b1[sl].reshape(2, 128).T),
            "vbias": np.broadcast_to(b1[sl], (128, ES)).copy(),
            "onesb": onesb,
        })

    res = run_bass_kernel_spmd(
        nc, in_maps, core_ids=list(range(N_CORES)),
        trace=bool(_CACHE.get("trace", False)))
    _CACHE["last_results"] = res

    output = np.empty((B, L, EMBED), dtype=np.float32)
    for b in range(B):
        acc = res.results[b * HPC]["out"].astype(np.float32).copy()
        for hg in range(1, HPC):
            acc += res.results[b * HPC + hg]["out"]
        output[b] = acc + b1[None, :]
    return output


# revision 8
# speedup vs baseline: 1.2230x; 1.2230x over previous
"""Multi-headed attention (B=2, L=2048, E=1024, H=16) on 8 trn2 cores.

Sharding: batch (2) x head-groups (4) -> 8 cores. Each core computes 4 heads
of one batch element end-to-end (QKV projection, attention, partial output
projection); host sums the 4 per-head-group partial outputs per batch and
adds the final bias.

All matmuls run in float32r (full-rate fp32 with reduced internal precision).
Layout avoids every transpose except V (PE-transposed once):
  - Q^T, K^T computed directly as [e, l] (head dim on partitions).
  - Scores computed k-major (S^T) in 1024-wide PSUM tiles; exp on ACT goes
    straight PSUM -> SBUF(f32r) in one wide instruction.
  - V stored interleaved [k, 4*(64+1)] with a ones column per head, so the
    PV matmul also accumulates the softmax denominators for free.
  - Normalization: DVE reciprocal + GPSIMD partition_broadcast + DVE mul.
  - ctx^T [e, l] is exactly the lhsT the output projection needs.
Emission order interleaves Q-projection chunks with attention q-chunk pairs
so the PE never queues behind DMA-gated projection work it doesn't need yet.
"""

import numpy as np

EMBED = 1024
HEADS = 16
HD = 64
B = 2
L = 2048
N_CORES = 8
HPC = 4              # heads per core
ES = HPC * HD        # 256: e-slice width per core
NEC = EMBED // 128   # 8 embed chunks
NQC = L // 512       # 4 q-chunks (projection granularity)
NQP = L // 1024      # 2 q-chunk-pairs (attention granularity)
NKT = L // 128       # 16 k-tiles
VW = HPC * (HD + 1)  # 260: interleaved V width

_CACHE = {}


def _gen_kernel():
    from contextlib import ExitStack

    import concourse.mybir as mybir
    import concourse.tile as tile
    from concourse import bacc
    from concourse.masks import make_identity

    dt = mybir.dt
    f32 = dt.float32
    f32r = dt.float32r

    nc = bacc.Bacc("TRN2", target_bir_lowering=False)

    xqT = nc.dram_tensor("xqT", [EMBED, L], f32, kind="ExternalInput")
    xkT = nc.dram_tensor("xkT", [EMBED, L], f32, kind="ExternalInput")
    xvT = nc.dram_tensor("xvT", [EMBED, L], f32, kind="ExternalInput")
    wT = nc.dram_tensor("wT", [EMBED, ES], f32, kind="ExternalInput")
    woT = nc.dram_tensor("woT", [ES, EMBED], f32, kind="ExternalInput")
    bqkv = nc.dram_tensor("bqkv", [128, 2], f32, kind="ExternalInput")
    vbias = nc.dram_tensor("vbias", [128, ES], f32, kind="ExternalInput")
    onesb = nc.dram_tensor("onesb", [128, 64], f32, kind="ExternalInput")
    out = nc.dram_tensor("out", [L, EMBED], f32, kind="ExternalOutput")

    with tile.TileContext(nc) as tc, ExitStack() as ctx:
        const = ctx.enter_context(tc.tile_pool(name="const", bufs=1))
        stage = ctx.enter_context(tc.tile_pool(name="stage", bufs=8))
        xrnd = ctx.enter_context(tc.tile_pool(name="xrnd", bufs=12))
        big = ctx.enter_context(tc.tile_pool(name="big", bufs=1))
        ptp = ctx.enter_context(tc.tile_pool(name="ptp", bufs=3))
        misc = ctx.enter_context(tc.tile_pool(name="misc", bufs=2))
        opool = ctx.enter_context(tc.tile_pool(name="opool", bufs=3))
        # PSUM budget (8 banks): pp_s 2x2 + pp_ctx 2x2 = 8
        pp_s = ctx.enter_context(tc.tile_pool(name="pp_s", bufs=2, space="PSUM"))
        pp_ctx = ctx.enter_context(tc.tile_pool(name="pp_ctx", bufs=2, space="PSUM"))

        # ---- constants ---------------------------------------------------
        wt_f = stage.tile([128, NEC * ES], f32, tag="wstage", bufs=1)
        for c in range(NEC):
            nc.sync.dma_start(wt_f[:, c * ES:(c + 1) * ES], wT[c * 128:(c + 1) * 128, :])
        wt_r = const.tile([128, NEC * ES], f32r)
        nc.vector.tensor_copy(wt_r[:], wt_f[:])

        wo_f = stage.tile([128, 2 * EMBED], f32, tag="wstage", bufs=1)
        for g in range(2):
            nc.sync.dma_start(wo_f[:, g * EMBED:(g + 1) * EMBED], woT[g * 128:(g + 1) * 128, :])
        wo_r = const.tile([128, 2 * EMBED], f32r)
        nc.vector.tensor_copy(wo_r[:], wo_f[:])

        bq = const.tile([128, 2], f32)
        nc.sync.dma_start(bq[:], bqkv[:])
        vb = const.tile([128, ES], f32)
        nc.sync.dma_start(vb[:], vbias[:])
        ones_f = const.tile([128, 64], f32)
        nc.sync.dma_start(ones_f[:], onesb[:])

        idn = const.tile([128, 128], f32)
        make_identity(nc, idn[:])

        # ---- persistent activations --------------------------------------
        # qt_p[qcp]: [e-group g at free g*1024, local l 0:1024]
        qt_p = [big.tile([128, 2048], f32r, tag=f"qtp{i}", name=f"qtp{i}") for i in range(NQP)]
        ktt = big.tile([128, 2 * L], f32r, tag="ktt")
        vtt = big.tile([128, 2 * L], f32, tag="vtt")
        vaug = big.tile([128, NKT * VW], f32r, tag="vaug")
        ctx_p = [big.tile([128, 2048], f32r, tag=f"ctxp{i}", name=f"ctxp{i}") for i in range(NQP)]

        def project(xdram, qc, write):
            """One 512-wide q-chunk of a projection: load x^T chunks, round,
            two 8-step accumulation chains (e-groups), evict via `write`."""
            blocks = []
            for c in range(NEC):
                bf = stage.tile([128, 512], f32, tag="xblk")
                nc.sync.dma_start(
                    bf[:], xdram[c * 128:(c + 1) * 128, qc * 512:(qc + 1) * 512])
                br = xrnd.tile([128, 512], f32r, tag="xrnd")
                nc.vector.tensor_copy(br[:], bf[:])
                blocks.append(br)
            for g in range(2):
                ps = pp_s.tile([128, 1024], f32, tag="s")
                for c in range(NEC):
                    nc.tensor.matmul(
                        ps[:, 0:512],
                        lhsT=wt_r[:, c * ES + g * 128: c * ES + (g + 1) * 128],
                        rhs=blocks[c][:],
                        start=(c == 0),
                        stop=(c == NEC - 1),
                    )
                write(g, qc, ps)

        def qkv_write(dest_of_gqc):
            def _w(g, qc, ps):
                nc.vector.tensor_scalar_add(
                    dest_of_gqc(g, qc), ps[:, 0:512], bq[:, g:g + 1])
            return _w

        # ---- V projection + interleave -----------------------------------
        for qc in range(NQC):
            project(xvT, qc, qkv_write(
                lambda g, qc: vtt[:, g * L + qc * 512: g * L + (qc + 1) * 512]))
        for t in range(NKT):
            for g in range(2):
                tp = pp_s.tile([128, 1024], f32, tag="s")
                nc.tensor.transpose(
                    tp[:, 0:128], vtt[:, g * L + t * 128: g * L + (t + 1) * 128], idn[:])
                o0 = t * VW + (2 * g) * (HD + 1)
                dst = vaug[:, o0: o0 + 2 * (HD + 1)].rearrange(
                    "p (a b) -> p a b", b=HD + 1)[:, :, 0:HD]
                nc.vector.tensor_add(
                    dst,
                    tp[:, 0:128].rearrange("p (a b) -> p a b", a=2),
                    vb[:, g * 128:(g + 1) * 128].rearrange("p (a b) -> p a b", a=2),
                )
        ones_dst = vaug[:].rearrange(
            "p (t h x) -> p x (t h)", h=HPC, x=HD + 1)[:, HD:HD + 1, :]
        nc.vector.tensor_copy(
            ones_dst, ones_f[:, 0:NKT * HPC].rearrange("p (a b) -> p a b", a=1))

        # ---- K projection ------------------------------------------------
        for qc in range(NQC):
            project(xkT, qc, qkv_write(
                lambda g, qc: ktt[:, g * L + qc * 512: g * L + (qc + 1) * 512]))

        inv_sqrt_e = 1.0 / 32.0

        def attention(qcp):
            """All 4 heads for one 1024-wide q-chunk-pair, SW-pipelined."""
            qtile = qt_p[qcp]
            for h in range(HPC):
                g = h // 2
                off = (h % 2) * 64

                def s_mm(kt, sps):
                    for half in range(2):
                        nc.tensor.matmul(
                            sps[:, half * 512:(half + 1) * 512],
                            lhsT=ktt[off:off + 64,
                                     g * L + kt * 128: g * L + (kt + 1) * 128],
                            rhs=qtile[off:off + 64,
                                      g * 1024 + half * 512: g * 1024 + (half + 1) * 512],
                            start=True,
                            stop=True,
                        )

                cps = pp_ctx.tile([128, 1024], f32, tag="ctx")
                sps_list = [pp_s.tile([128, 1024], f32, tag="s", name="sps0")]
                s_mm(0, sps_list[0])
                for kt in range(NKT):
                    if kt + 1 < NKT:
                        nxt = pp_s.tile([128, 1024], f32, tag="s")
                        sps_list.append(nxt)
                        s_mm(kt + 1, nxt)
                    sps = sps_list[kt]
                    pt = ptp.tile([128, 1024], f32r, tag="pt")
                    nc.scalar.activation(
                        pt[:], sps[:], mybir.ActivationFunctionType.Exp,
                        scale=inv_sqrt_e)
                    for half in range(2):
                        nc.tensor.matmul(
                            cps[0:65, half * 512:(half + 1) * 512],
                            lhsT=vaug[:, kt * VW + h * (HD + 1): kt * VW + (h + 1) * (HD + 1)],
                            rhs=pt[:, half * 512:(half + 1) * 512],
                            start=(kt == 0),
                            stop=(kt == NKT - 1),
                        )
                # normalize rows 0:64 by the denominator row 64
                rec = misc.tile([128, 1024], f32, tag="rec")
                nc.vector.reciprocal(rec[0:1, :], cps[64:65, :])
                bcs = misc.tile([128, 1024], f32, tag="bcs")
                nc.gpsimd.partition_broadcast(bcs[0:64, :], rec[0:1, :])
                nc.vector.tensor_mul(
                    ctx_p[qcp][off:off + 64, g * 1024:(g + 1) * 1024],
                    cps[0:64, :],
                    bcs[0:64, :],
                )

        def out_proj(qcp):
            for lt8 in range(8):
                for oc in range(2):
                    ops = pp_ctx.tile([128, 1024], f32, tag="ctx")
                    for g in range(2):
                        nc.tensor.matmul(
                            ops[:, 0:512],
                            lhsT=ctx_p[qcp][:, g * 1024 + lt8 * 128: g * 1024 + (lt8 + 1) * 128],
                            rhs=wo_r[:, g * EMBED + oc * 512: g * EMBED + (oc + 1) * 512],
                            start=(g == 0),
                            stop=(g == 1),
                        )
                    ot = opool.tile([128, 512], f32, tag="ot")
                    nc.vector.tensor_copy(ot[:], ops[:, 0:512])
                    lt = qcp * 8 + lt8
                    nc.sync.dma_start(
                        out[lt * 128:(lt + 1) * 128, oc * 512:(oc + 1) * 512], ot[:])

        # ---- Q projection interleaved with attention ---------------------
        for qcp in range(NQP):
            for qc in (2 * qcp, 2 * qcp + 1):
                project(xqT, qc, qkv_write(
                    lambda g, qc: qt_p[qc // 2][
                        :, g * 1024 + (qc % 2) * 512: g * 1024 + (qc % 2 + 1) * 512]))
            attention(qcp)
            out_proj(qcp)

    nc.compile()
    return nc


def kernel(query, key, values, W1, b1):
    from concourse.bass_utils import run_bass_kernel_spmd

    if "nc" not in _CACHE:
        _CACHE["nc"] = _gen_kernel()
    nc = _CACHE["nc"]

    query = np.asarray(query, dtype=np.float32)
    key = np.asarray(key, dtype=np.float32)
    values = np.asarray(values, dtype=np.float32)
    W1 = np.asarray(W1, dtype=np.float32)
    b1 = np.asarray(b1, dtype=np.float32)

    xT = {}
    for b in range(B):
        xT[("q", b)] = np.ascontiguousarray(query[b].T)
        xT[("k", b)] = np.ascontiguousarray(key[b].T)
        xT[("v", b)] = np.ascontiguousarray(values[b].T)

    onesb = np.ones((128, 64), dtype=np.float32)
    in_maps = []
    for core in range(N_CORES):
        b = core // HPC
        hg = core % HPC
        sl = slice(hg * ES, (hg + 1) * ES)
        in_maps.append({
            "xqT": xT[("q", b)],
            "xkT": xT[("k", b)],
            "xvT": xT[("v", b)],
            "wT": np.ascontiguousarray(W1[sl, :].T),
            "woT": np.ascontiguousarray(W1[:, sl].T),
            "bqkv": np.ascontiguousarray(b1[sl].reshape(2, 128).T),
            "vbias": np.broadcast_to(b1[sl], (128, ES)).copy(),
            "onesb": onesb,
        })

    res = run_bass_kernel_spmd(
        nc, in_maps, core_ids=list(range(N_CORES)),
        trace=bool(_CACHE.get("trace", False)))
    _CACHE["last_results"] = res

    output = np.empty((B, L, EMBED), dtype=np.float32)
    for b in range(B):
        acc = res.results[b * HPC]["out"].astype(np.float32).copy()
        for hg in range(1, HPC):
            acc += res.results[b * HPC + hg]["out"]
        output[b] = acc + b1[None, :]
    return output


# revision 9
# speedup vs baseline: 1.2542x; 1.0255x over previous
"""Multi-headed attention (B=2, L=2048, E=1024, H=16) on 8 trn2 cores.

Sharding: batch (2) x head-groups (4) -> 8 cores. Each core computes 4 heads
of one batch element end-to-end (QKV projection, attention, partial output
projection); host sums the 4 per-head-group partial outputs per batch and
adds the final bias.

All matmuls run in float32r (full-rate fp32 with reduced internal precision).
Layout avoids every transpose except V (PE-transposed once):
  - Q^T, K^T computed directly as [e, l] (head dim on partitions).
  - Scores computed k-major (S^T) in 1024-wide PSUM tiles; exp on ACT goes
    straight PSUM -> SBUF(f32r) in one wide instruction.
  - V stored interleaved [k, 4*(64+1)] with a ones column per head, so the
    PV matmul also accumulates the softmax denominators for free.
  - Normalization: DVE reciprocal + GPSIMD partition_broadcast + DVE mul.
  - ctx^T [e, l] is exactly the lhsT the output projection needs.
Emission order interleaves Q-projection chunks with attention q-chunk pairs
so the PE never queues behind DMA-gated projection work it doesn't need yet.
"""

import numpy as np

EMBED = 1024
HEADS = 16
HD = 64
B = 2
L = 2048
N_CORES = 8
HPC = 4              # heads per core
ES = HPC * HD        # 256: e-slice width per core
NEC = EMBED // 128   # 8 embed chunks
NQC = L // 512       # 4 q-chunks (projection granularity)
NQP = L // 1024      # 2 q-chunk-pairs (attention granularity)
NKT = L // 128       # 16 k-tiles
VW = HPC * (HD + 1)  # 260: interleaved V width

_CACHE = {}


def _gen_kernel():
    from contextlib import ExitStack

    import concourse.mybir as mybir
    import concourse.tile as tile
    from concourse import bacc
    from concourse.masks import make_identity

    dt = mybir.dt
    f32 = dt.float32
    f32r = dt.float32r

    nc = bacc.Bacc("TRN2", target_bir_lowering=False)

    xqT = nc.dram_tensor("xqT", [EMBED, L], f32, kind="ExternalInput")
    xkT = nc.dram_tensor("xkT", [EMBED, L], f32, kind="ExternalInput")
    xvT = nc.dram_tensor("xvT", [EMBED, L], f32, kind="ExternalInput")
    wT = nc.dram_tensor("wT", [EMBED, ES], f32, kind="ExternalInput")
    woT = nc.dram_tensor("woT", [ES, EMBED], f32, kind="ExternalInput")
    bqkv = nc.dram_tensor("bqkv", [128, 2], f32, kind="ExternalInput")
    vbias = nc.dram_tensor("vbias", [128, ES], f32, kind="ExternalInput")
    onesb = nc.dram_tensor("onesb", [128, 64], f32, kind="ExternalInput")
    out = nc.dram_tensor("out", [L, EMBED], f32, kind="ExternalOutput")

    with tile.TileContext(nc) as tc, ExitStack() as ctx:
        const = ctx.enter_context(tc.tile_pool(name="const", bufs=1))
        stage = ctx.enter_context(tc.tile_pool(name="stage", bufs=8))
        xrnd = ctx.enter_context(tc.tile_pool(name="xrnd", bufs=12))
        big = ctx.enter_context(tc.tile_pool(name="big", bufs=1))
        ptp = ctx.enter_context(tc.tile_pool(name="ptp", bufs=3))
        misc = ctx.enter_context(tc.tile_pool(name="misc", bufs=2))
        opool = ctx.enter_context(tc.tile_pool(name="opool", bufs=3))
        # PSUM budget (8 banks): pp_s 2x2 + pp_ctx 2x2 = 8
        pp_s = ctx.enter_context(tc.tile_pool(name="pp_s", bufs=2, space="PSUM"))
        pp_ctx = ctx.enter_context(tc.tile_pool(name="pp_ctx", bufs=2, space="PSUM"))

        # ---- constants ---------------------------------------------------
        wt_f = stage.tile([128, NEC * ES], f32, tag="wstage", bufs=1)
        for c in range(NEC):
            nc.sync.dma_start(wt_f[:, c * ES:(c + 1) * ES], wT[c * 128:(c + 1) * 128, :])
        wt_r = const.tile([128, NEC * ES], f32r)
        nc.vector.tensor_copy(wt_r[:], wt_f[:])

        wo_f = stage.tile([128, 2 * EMBED], f32, tag="wstage", bufs=1)
        for g in range(2):
            nc.sync.dma_start(wo_f[:, g * EMBED:(g + 1) * EMBED], woT[g * 128:(g + 1) * 128, :])
        wo_r = const.tile([128, 2 * EMBED], f32r)
        nc.vector.tensor_copy(wo_r[:], wo_f[:])

        bq = const.tile([128, 2], f32)
        nc.sync.dma_start(bq[:], bqkv[:])
        vb = const.tile([128, ES], f32)
        nc.sync.dma_start(vb[:], vbias[:])
        ones_f = const.tile([128, 64], f32)
        nc.sync.dma_start(ones_f[:], onesb[:])

        idn = const.tile([128, 128], f32)
        make_identity(nc, idn[:])

        # ---- persistent activations --------------------------------------
        # qt_p[qcp]: [e-group g at free g*1024, local l 0:1024]
        qt_p = [big.tile([128, 2048], f32r, tag=f"qtp{i}", name=f"qtp{i}") for i in range(NQP)]
        ktt_q = [big.tile([128, 1024], f32r, tag=f"kttq{i}", name=f"kttq{i}")
                 for i in range(NQC)]
        vtt_q = [big.tile([128, 1024], f32, tag=f"vttq{i}", name=f"vttq{i}")
                 for i in range(NQC)]
        vaug_q = [big.tile([128, 4 * VW], f32r, tag=f"vaugq{i}", name=f"vaugq{i}")
                  for i in range(NQC)]
        ctx_p = [big.tile([128, 2048], f32r, tag=f"ctxp{i}", name=f"ctxp{i}") for i in range(NQP)]

        def project(xdram, qc, write):
            """One 512-wide q-chunk of a projection: load x^T chunks, round,
            two 8-step accumulation chains (e-groups), evict via `write`."""
            blocks = []
            for c in range(NEC):
                bf = stage.tile([128, 512], f32, tag="xblk")
                nc.sync.dma_start(
                    bf[:], xdram[c * 128:(c + 1) * 128, qc * 512:(qc + 1) * 512])
                br = xrnd.tile([128, 512], f32r, tag="xrnd")
                nc.vector.tensor_copy(br[:], bf[:])
                blocks.append(br)
            for g in range(2):
                ps = pp_s.tile([128, 1024], f32, tag="s")
                for c in range(NEC):
                    nc.tensor.matmul(
                        ps[:, 0:512],
                        lhsT=wt_r[:, c * ES + g * 128: c * ES + (g + 1) * 128],
                        rhs=blocks[c][:],
                        start=(c == 0),
                        stop=(c == NEC - 1),
                    )
                write(g, qc, ps)

        def qkv_write(dest_of_gqc):
            def _w(g, qc, ps):
                nc.vector.tensor_scalar_add(
                    dest_of_gqc(g, qc), ps[:, 0:512], bq[:, g:g + 1])
            return _w

        # ---- V+K projections streamed per quarter ------------------------
        for qc in range(NQC):
            project(xvT, qc, qkv_write(
                lambda g, qc: vtt_q[qc][:, g * 512:(g + 1) * 512]))
            for j in range(4):
                for g in range(2):
                    tp = pp_s.tile([128, 1024], f32, tag="s")
                    nc.tensor.transpose(
                        tp[:, 0:128],
                        vtt_q[qc][:, g * 512 + j * 128: g * 512 + (j + 1) * 128],
                        idn[:])
                    o0 = j * VW + (2 * g) * (HD + 1)
                    dst = vaug_q[qc][:, o0: o0 + 2 * (HD + 1)].rearrange(
                        "p (a b) -> p a b", b=HD + 1)[:, :, 0:HD]
                    nc.vector.tensor_add(
                        dst,
                        tp[:, 0:128].rearrange("p (a b) -> p a b", a=2),
                        vb[:, g * 128:(g + 1) * 128].rearrange("p (a b) -> p a b", a=2),
                    )
            ones_dst = vaug_q[qc][:].rearrange(
                "p (t h x) -> p x (t h)", h=HPC, x=HD + 1)[:, HD:HD + 1, :]
            nc.vector.tensor_copy(
                ones_dst, ones_f[:, 0:4 * HPC].rearrange("p (a b) -> p a b", a=1))
            project(xkT, qc, qkv_write(
                lambda g, qc: ktt_q[qc][:, g * 512:(g + 1) * 512]))

        inv_sqrt_e = 1.0 / 32.0

        def attention(qcp):
            """All 4 heads for one 1024-wide q-chunk-pair, SW-pipelined."""
            qtile = qt_p[qcp]
            for h in range(HPC):
                g = h // 2
                off = (h % 2) * 64

                def s_mm(kt, sps):
                    for half in range(2):
                        nc.tensor.matmul(
                            sps[:, half * 512:(half + 1) * 512],
                            lhsT=ktt_q[kt // 4][
                                off:off + 64,
                                g * 512 + (kt % 4) * 128: g * 512 + (kt % 4 + 1) * 128],
                            rhs=qtile[off:off + 64,
                                      g * 1024 + half * 512: g * 1024 + (half + 1) * 512],
                            start=True,
                            stop=True,
                        )

                cps = pp_ctx.tile([128, 1024], f32, tag="ctx")
                sps_list = [pp_s.tile([128, 1024], f32, tag="s", name="sps0")]
                s_mm(0, sps_list[0])
                for kt in range(NKT):
                    if kt + 1 < NKT:
                        nxt = pp_s.tile([128, 1024], f32, tag="s")
                        sps_list.append(nxt)
                        s_mm(kt + 1, nxt)
                    sps = sps_list[kt]
                    pt = ptp.tile([128, 1024], f32r, tag="pt")
                    nc.scalar.activation(
                        pt[:], sps[:], mybir.ActivationFunctionType.Exp,
                        scale=inv_sqrt_e)
                    for half in range(2):
                        nc.tensor.matmul(
                            cps[0:65, half * 512:(half + 1) * 512],
                            lhsT=vaug_q[kt // 4][
                                :, (kt % 4) * VW + h * (HD + 1): (kt % 4) * VW + (h + 1) * (HD + 1)],
                            rhs=pt[:, half * 512:(half + 1) * 512],
                            start=(kt == 0),
                            stop=(kt == NKT - 1),
                        )
                # normalize rows 0:64 by the denominator row 64
                rec = misc.tile([128, 1024], f32, tag="rec")
                nc.vector.reciprocal(rec[0:1, :], cps[64:65, :])
                bcs = misc.tile([128, 1024], f32, tag="bcs")
                nc.gpsimd.partition_broadcast(bcs[0:64, :], rec[0:1, :])
                nc.vector.tensor_mul(
                    ctx_p[qcp][off:off + 64, g * 1024:(g + 1) * 1024],
                    cps[0:64, :],
                    bcs[0:64, :],
                )

        def out_proj(qcp):
            for lt8 in range(8):
                for oc in range(2):
                    ops = pp_ctx.tile([128, 1024], f32, tag="ctx")
                    for g in range(2):
                        nc.tensor.matmul(
                            ops[:, 0:512],
                            lhsT=ctx_p[qcp][:, g * 1024 + lt8 * 128: g * 1024 + (lt8 + 1) * 128],
                            rhs=wo_r[:, g * EMBED + oc * 512: g * EMBED + (oc + 1) * 512],
                            start=(g == 0),
                            stop=(g == 1),
                        )
                    ot = opool.tile([128, 512], f32, tag="ot")
                    nc.vector.tensor_copy(ot[:], ops[:, 0:512])
                    lt = qcp * 8 + lt8
                    nc.sync.dma_start(
                        out[lt * 128:(lt + 1) * 128, oc * 512:(oc + 1) * 512], ot[:])

        # ---- Q projection interleaved with attention ---------------------
        def qproj(qc):
            project(xqT, qc, qkv_write(
                lambda g, qc: qt_p[qc // 2][
                    :, g * 1024 + (qc % 2) * 512: g * 1024 + (qc % 2 + 1) * 512]))

        qproj(0)
        qproj(1)
        attention(0)
        qproj(2)
        qproj(3)
        attention(1)
        out_proj(0)
        out_proj(1)

    nc.compile()
    return nc


def kernel(query, key, values, W1, b1):
    from concourse.bass_utils import run_bass_kernel_spmd

    if "nc" not in _CACHE:
        _CACHE["nc"] = _gen_kernel()
    nc = _CACHE["nc"]

    query = np.asarray(query, dtype=np.float32)
    key = np.asarray(key, dtype=np.float32)
    values = np.asarray(values, dtype=np.float32)
    W1 = np.asarray(W1, dtype=np.float32)
    b1 = np.asarray(b1, dtype=np.float32)

    xT = {}
    for b in range(B):
        xT[("q", b)] = np.ascontiguousarray(query[b].T)
        xT[("k", b)] = np.ascontiguousarray(key[b].T)
        xT[("v", b)] = np.ascontiguousarray(values[b].T)

    onesb = np.ones((128, 64), dtype=np.float32)
    in_maps = []
    for core in range(N_CORES):
        b = core // HPC
        hg = core % HPC
        sl = slice(hg * ES, (hg + 1) * ES)
        in_maps.append({
            "xqT": xT[("q", b)],
            "xkT": xT[("k", b)],
            "xvT": xT[("v", b)],
            "wT": np.ascontiguousarray(W1[sl, :].T),
            "woT": np.ascontiguousarray(W1[:, sl].T),
            "bqkv": np.ascontiguousarray(b1[sl].reshape(2, 128).T),
            "vbias": np.broadcast_to(b1[sl], (128, ES)).copy(),
            "onesb": onesb,
        })

    res = run_bass_kernel_spmd(
        nc, in_maps, core_ids=list(range(N_CORES)),
        trace=bool(_CACHE.get("trace", False)))
    _CACHE["last_results"] = res

    output = np.empty((B, L, EMBED), dtype=np.float32)
    for b in range(B):
        acc = res.results[b * HPC]["out"].astype(np.float32).copy()
        for hg in range(1, HPC):
            acc += res.results[b * HPC + hg]["out"]
        output[b] = acc + b1[None, :]
    return output
